# revision 8
# baseline (speedup 1.0000x reference)
"""JointVAE (vq_codebook) forward pass on 8 Trainium2 NeuronCores.

Data-parallel: batch dim (8192) sharded 1024 rows/core; all MLP weights and
the [K,D] codebook replicated. Everything per-core runs in one Bass/Tile
program: encoders -> VQ (argmin via max8/max_index on -dist surrogate,
codebook row gather via indirect DMA) -> 4 decoder passes.

Matmuls run in float32r (TF32-like, full PE rate). Activations are kept
transposed (hT: [D on partitions, batch free]) so residual blocks / VQ
score matmuls need no per-tile transposes; x is transposed on the PE.
"""
import sys

sys.path.insert(0, "/opt/trn_rl_repo")

import json

import numpy as np

import concourse.bass as bass
import concourse.mybir as mybir
import concourse.tile as tile
from concourse.bass_utils import run_bass_kernel_spmd
from concourse.vector_clock import VectorClock, ScopedClock

# ----- problem constants (hardcoded; kernel.py must be self-contained) -----
B = 8192
DIN_A, DIN_B = 4096, 2048
D = 256          # latent dim
K = 8192         # codebook entries
NRES = 2         # residual blocks
BETA = 0.25
NCORES = 8
BC = B // NCORES  # rows per core = 1024

P = 128
R = mybir.dt.float32r
F = mybir.dt.float32
U32 = mybir.dt.uint32
ACTF = mybir.ActivationFunctionType


class _SplitDrainTileContext(tile.TileContext):
    """Walrus in this env allows only 1 sem-wait per CTRL instruction.
    Absorb the tail-drain's global-clock waits through a chain of SP nops
    (1 wait each); the drain then needs none (SP executes in order)."""

    def _drain_and_barrier(self, tick_clock, wait_clock):
        gc = tick_clock.global_clock
        for p in range(len(gc)):
            if gc[p] <= 0:
                continue
            nop_inst = self.nc.sync.nop(nofuse=True).ins
            partial = VectorClock()
            partial.require_at_least(p, gc[p])
            wait_clock.add_sem_waits(nop_inst, ScopedClock({None: partial}))
        self.nc.sync.drain()
        self.nc.all_engine_barrier()
        assert self.sems is not None
        popped = self.nc._tile_sem_poison_stack.pop()
        assert popped is self._sem_poison
        self.nc.clear_and_free_semaphores(list(self.sems.allocated().values()))
        self.nc.all_engine_barrier()


def _split_waits_json(data: bytes, max_waits: int = 1) -> bytes:
    """Split any instruction carrying > max_waits sem-waits into preceding
    NoOps on the same engine (walrus here rejects multi-wait instructions)."""
    d = json.loads(data)
    n = 0
    for f in d["functions"]:
        for bb in f["blocks"]:
            out = []
            changed = False
            for inst in bb["instructions"]:
                si = inst.get("sync_info")
                waits = (si or {}).get("on_wait") or []
                if len(waits) > max_waits:
                    changed = True
                    extras, keep = waits[:-max_waits], waits[-max_waits:]
                    for i in range(0, len(extras), max_waits):
                        n += 1
                        out.append({
                            "debug": inst.get("debug", 0),
                            "engine": inst["engine"],
                            "ins": [], "outs": [],
                            "name": f"WSPLIT-{n}",
                            "opcode": "NoOp",
                            "sync_info": {
                                "on_update": [],
                                "on_wait": extras[i:i + max_waits],
                            },
                        })
                    si["on_wait"] = keep
                out.append(inst)
            if changed:
                bb["instructions"] = out
    return json.dumps(d).encode()


class _PatchedBass(bass.Bass):
    def to_json_bytes(self, *a, **k):
        return _split_waits_json(super().to_json_bytes(*a, **k))


def _encoder_phase(nc, tc, pools, x_dram, w_sb, bias_sb, rw_sb, rb_sb, din, zT):
    """relu(x @ W + b) then NRES residual blocks, all in T layout.
    Writes zT [128, 2*BC] (half-major columns)."""
    sb = pools["enc_sb"]
    ps = pools["enc_ps"]
    ident = pools["identF"]
    nch = din // P
    GE = 256                      # batch group for the input projection
    for g in range(BC // GE):
        xs = []
        for sub in range(2):
            t = sb.tile([P, din], F, tag="xstage")
            nc.sync.dma_start(t[:], x_dram[g * GE + sub * P: g * GE + (sub + 1) * P, :])
            xs.append(t)
        ph = [ps.tile([P, GE], F, space="PSUM", tag=f"pAB{h}", name=f"ph{h}") for h in range(2)]
        for c in range(nch):
            pt = ps.tile([P, GE], F, space="PSUM", tag="xtp")
            for sub in range(2):
                nc.tensor.transpose(out=pt[:, sub * P:(sub + 1) * P],
                                    in_=xs[sub][:, c * P:(c + 1) * P],
                                    identity=ident[:])
            xtc = sb.tile([P, GE], F, tag="xtc")
            nc.vector.tensor_copy(out=xtc[:], in_=pt[:])
            for h in range(2):
                nc.tensor.matmul(out=ph[h][:],
                                 lhsT=w_sb[:, c * D + h * P: c * D + (h + 1) * P],
                                 rhs=xtc[:],
                                 start=(c == 0), stop=(c == nch - 1))
        for h in range(2):
            nc.scalar.activation(out=zT[:, h * BC + g * GE: h * BC + (g + 1) * GE],
                                 in_=ph[h][:], func=ACTF.Relu,
                                 bias=bias_sb[:, h:h + 1])
    _res_blocks(nc, tc, pools, rw_sb, rb_sb, zT, zT, dt=F)


def _res_blocks(nc, tc, pools, rw_sb, rb_sb, src, dst, dt):
    """dst = src after NRES blocks of h = h + relu(h @ rw[i] + rb[i]).
    src/dst are [128, 2*BC] T-layout tiles; src may equal dst."""
    sb = pools["enc_sb"]
    ps = pools["enc_ps"]
    G2 = 512
    cur = src
    for r in range(NRES):
        for g in range(BC // G2):
            pr = [ps.tile([P, G2], F, space="PSUM", tag=f"pAB{h}", name=f"pr{h}") for h in range(2)]
            for ho in range(2):
                for ci in range(2):
                    nc.tensor.matmul(
                        out=pr[ho][:],
                        lhsT=rw_sb[:, r * 2 * D + ci * D + ho * P:
                                   r * 2 * D + ci * D + (ho + 1) * P],
                        rhs=cur[:, ci * BC + g * G2: ci * BC + (g + 1) * G2],
                        start=(ci == 0), stop=(ci == 1))
            for ho in range(2):
                tmp = sb.tile([P, G2], dt, tag=f"rtmp{ho}")
                nc.scalar.activation(out=tmp[:], in_=pr[ho][:], func=ACTF.Relu,
                                     bias=rb_sb[:, r * 2 + ho: r * 2 + ho + 1])
                dsl = dst[:, ho * BC + g * G2: ho * BC + (g + 1) * G2]
                csl = cur[:, ho * BC + g * G2: ho * BC + (g + 1) * G2]
                nc.vector.tensor_add(out=dsl, in0=csl, in1=tmp[:])
        cur = dst
    return dst


def _vq_phase(nc, tc, pools, zT, ET2, E_dram, za_out, loss_out, zqT, idx_out):
    """Bucket-exact VQ: replicate the reference's fp32 rounding of
    dist = fl(fl(|z|^2 + |E_k|^2) - fl(2 z.E)) so argmin matches the fp32
    CPU reference (near-ties included). We compute neg-dist
    sc = fl(M - T1) with M = z @ (2E^T) from PSUM and T1 = fl(E2 + |z|^2)
    built on ACT from a broadcast E2 tile, then argmax via max8/max_index."""
    sb = pools["vq_sb"]
    sm = pools["vq_small"]
    ps = pools["vq_ps"]
    identF = pools["identF"]
    e2b = pools["e2b"]        # [128, K] fp32: |E_k|^2 broadcast to all rows
    for t in range(BC // P):
        # z tile (batch-major, exact fp32 transpose of zT) and |z|^2
        pz = ps.tile([P, D], F, space="PSUM", tag="pz")
        for ci in range(2):
            nc.tensor.transpose(out=pz[:, ci * P:(ci + 1) * P],
                                in_=zT[:, ci * BC + t * P: ci * BC + (t + 1) * P],
                                identity=identF[:])
        z = sm.tile([P, D], F, tag="z")
        nc.vector.tensor_copy(out=z[:], in_=pz[:])
        zsq = sm.tile([P, D], F, tag="zsq")
        z2 = sm.tile([P, 1], F, tag="z2")
        nc.scalar.activation(out=zsq[:], in_=z[:], func=ACTF.Square,
                             accum_out=z2[:])
        sc = sb.tile([P, K], F, tag="sc")
        for n in range(K // 512):
            pv = ps.tile([P, 512], F, space="PSUM", tag="pv")
            for ci in range(2):
                nc.tensor.matmul(out=pv[:],
                                 lhsT=zT[:, ci * BC + t * P: ci * BC + (t + 1) * P],
                                 rhs=ET2[ci][:, n * 512:(n + 1) * 512],
                                 start=(ci == 0), stop=(ci == 1))
            # T1 = fl(E2 + z2) on ACT, then sc = fl(M - T1) on DVE (one
            # rounding each, mirroring the reference expression).
            nc.scalar.activation(out=sc[:, n * 512:(n + 1) * 512],
                                 in_=e2b[:, n * 512:(n + 1) * 512],
                                 func=ACTF.Identity, bias=z2[:, 0:1])
            nc.vector.tensor_tensor(out=sc[:, n * 512:(n + 1) * 512],
                                    in0=pv[:],
                                    in1=sc[:, n * 512:(n + 1) * 512],
                                    op=mybir.AluOpType.subtract)
        mx = sm.tile([P, 8], F, tag="mx")
        nc.vector.max(out=mx[:], in_=sc[:, :])
        mi = sm.tile([P, 8], U32, tag="mi")
        nc.vector.max_index(out=mi[:], in_max=mx[:], in_values=sc[:, :])
        nc.sync.dma_start(idx_out[t * P:(t + 1) * P, :], mi[:, 0:1])
        q = sm.tile([P, D], F, tag="q")
        nc.gpsimd.indirect_dma_start(
            out=q[:], out_offset=None, in_=E_dram[:, :],
            in_offset=bass.IndirectOffsetOnAxis(ap=mi[:, 0:1], axis=0))
        dtl = sm.tile([P, D], F, tag="dtl")
        nc.vector.tensor_sub(out=dtl[:], in0=q[:], in1=z[:])
        zq = sb.tile([P, D], F, tag="zq")
        nc.vector.tensor_add(out=zq[:], in0=z[:], in1=dtl[:])
        nc.sync.dma_start(za_out[t * P:(t + 1) * P, :], zq[:])
        sqt = sm.tile([P, D], F, tag="sqt")
        lac = sm.tile([P, 1], F, tag="lac")
        nc.scalar.activation(out=sqt[:], in_=dtl[:], func=ACTF.Square,
                             accum_out=lac[:])
        nc.vector.tensor_scalar_mul(lac[:], lac[:], 1.0 + BETA)
        nc.sync.dma_start(loss_out[t * P:(t + 1) * P, :], lac[:, 0:1])
        # z_q transposed (f32r) for the decoders
        pq = ps.tile([P, D], F, space="PSUM", tag="pz")
        for ci in range(2):
            nc.tensor.transpose(out=pq[:, ci * P:(ci + 1) * P],
                                in_=zq[:, ci * P:(ci + 1) * P],
                                identity=identF[:])
        for ci in range(2):
            nc.vector.tensor_copy(out=zqT[:, ci * BC + t * P: ci * BC + (t + 1) * P],
                                  in_=pq[:, ci * P:(ci + 1) * P])


def _decoder_phase(nc, tc, pools, zqT, rw_sb, rb_sb, w_sb, brow, dout, out_dram):
    """NRES residual blocks on zqT then final projection + bias, batch-major."""
    sb = pools["dec_sb"]
    ps = pools["dec_ps"]
    ones = pools["ones"]
    hT = sb.tile([P, 2 * BC], R, tag="hT")
    _res_blocks(nc, tc, {**pools, "enc_sb": sb, "enc_ps": ps},
                rw_sb, rb_sb, zqT, hT, dt=R)
    nch = dout // 512
    for t in range(BC // P):
        for ch in range(nch):
            pf = ps.tile([P, 512], F, space="PSUM", tag="pf")
            for ci in range(2):
                nc.tensor.matmul(out=pf[:],
                                 lhsT=hT[:, ci * BC + t * P: ci * BC + (t + 1) * P],
                                 rhs=w_sb[:, ci * dout + ch * 512: ci * dout + (ch + 1) * 512],
                                 start=(ci == 0), stop=False)
            nc.tensor.matmul(out=pf[:], lhsT=ones[:],
                             rhs=brow[:, ch * 512:(ch + 1) * 512],
                             start=False, stop=True)
            st = sb.tile([P, 512], F, tag="stg")
            nc.scalar.activation(out=st[:], in_=pf[:], func=ACTF.Copy)
            nc.sync.dma_start(out_dram[t * P:(t + 1) * P, ch * 512:(ch + 1) * 512],
                              st[:])


def _build_program():
    import os
    nc = _PatchedBass()
    dp = nc.declare_dram_parameter
    xa = dp("xa", [BC, DIN_A], F, isOutput=False)
    xb = dp("xb", [BC, DIN_B], F, isOutput=False)
    wa = dp("wa", [DIN_A, D], F, isOutput=False)
    wb = dp("wb", [DIN_B, D], F, isOutput=False)
    ba = dp("ba", [D], F, isOutput=False)
    bb = dp("bb", [D], F, isOutput=False)
    rwa = dp("rwa", [NRES, D, D], F, isOutput=False)
    rba = dp("rba", [NRES, D], F, isOutput=False)
    rwb = dp("rwb", [NRES, D, D], F, isOutput=False)
    rbb = dp("rbb", [NRES, D], F, isOutput=False)
    drwa = dp("drwa", [NRES, D, D], R, isOutput=False)
    drba = dp("drba", [NRES, D], F, isOutput=False)
    drwb = dp("drwb", [NRES, D, D], R, isOutput=False)
    drbb = dp("drbb", [NRES, D], F, isOutput=False)
    dwa = dp("dwa", [D, DIN_A], R, isOutput=False)
    dba = dp("dba", [DIN_A], R, isOutput=False)
    dwb = dp("dwb", [D, DIN_B], R, isOutput=False)
    dbb = dp("dbb", [DIN_B], R, isOutput=False)
    Ecb = dp("E", [K, D], F, isOutput=False)
    e2row = dp("e2", [1, K], F, isOutput=False)     # |E_k|^2 per entry
    identF_in = dp("identF", [P, P], F, isOutput=False)
    ones_in = dp("ones", [1, P], R, isOutput=False)

    ia = dp("ia", [BC, 1], U32, isOutput=True)
    ib = dp("ib", [BC, 1], U32, isOutput=True)
    za = dp("za", [BC, D], F, isOutput=True)
    zb = dp("zb", [BC, D], F, isOutput=True)
    la = dp("la", [BC, 1], F, isOutput=True)
    lb = dp("lb", [BC, 1], F, isOutput=True)
    ra = dp("ra", [BC, DIN_A], F, isOutput=True)
    rb_o = dp("rb", [BC, DIN_B], F, isOutput=True)
    ca = dp("ca", [BC, DIN_A], F, isOutput=True)
    cb = dp("cb", [BC, DIN_B], F, isOutput=True)

    with _SplitDrainTileContext(nc, trace_sim=bool(os.environ.get("VQ_TRACE"))) as tc:
        with tc.tile_pool(name="persist", bufs=1) as pp:
            pools = {}
            identF = pp.tile([P, P], F, tag="identF")
            nc.sync.dma_start(identF[:], identF_in[:, :])
            ones = pp.tile([1, P], R, tag="ones")
            nc.sync.dma_start(ones[:], ones_in[:, :])
            pools.update(identF=identF, ones=ones)

            def load_bias_pc(drm, n):      # [n*D] -> [128, n*D/128] (c p) -> p c
                t = pp.tile([P, n * D // P], F, tag=f"b_{drm.tensor.name}")
                nc.sync.dma_start(
                    t[:].rearrange("p c -> p c"),
                    drm.rearrange("(c p) -> p c", p=P))
                return t

            ba_sb = load_bias_pc(ba[:], 1)
            bb_sb = load_bias_pc(bb[:], 1)
            rba_sb = load_bias_pc(rba.rearrange("r d -> (r d)"), NRES)
            rbb_sb = load_bias_pc(rbb.rearrange("r d -> (r d)"), NRES)
            drba_sb = load_bias_pc(drba.rearrange("r d -> (r d)"), NRES)
            drbb_sb = load_bias_pc(drbb.rearrange("r d -> (r d)"), NRES)

            def load_w(drm, din, dout, name, pool=None, dt=R):
                t = (pool or pp).tile([P, (din // P) * dout], dt, tag=f"w_{name}",
                                      name=f"w_{name}")
                nc.sync.dma_start(
                    t[:].rearrange("p (c o) -> p c o", o=dout),
                    drm.rearrange("(c p) o -> p c o", p=P))
                return t

            zTa = pp.tile([P, 2 * BC], F, tag="zTa")
            zTb = pp.tile([P, 2 * BC], F, tag="zTb")
            zqTa = pp.tile([P, 2 * BC], R, tag="zqTa")
            zqTb = pp.tile([P, 2 * BC], R, tag="zqTb")

            # ---------------- encoders ----------------
            with tc.tile_pool(name="enc_sb", bufs=2) as enc_sb, \
                 tc.tile_pool(name="enc_ps", bufs=2, space="PSUM") as enc_ps, \
                 tc.tile_pool(name="enc_w", bufs=1) as enc_w:
                pools.update(enc_sb=enc_sb, enc_ps=enc_ps)
                rwa_sb = load_w(rwa.rearrange("r i o -> (r i) o"), NRES * D, D,
                                "rwa", pool=enc_w, dt=F)
                rwb_sb = load_w(rwb.rearrange("r i o -> (r i) o"), NRES * D, D,
                                "rwb", pool=enc_w, dt=F)
                wa_sb = enc_w.tile([P, (DIN_A // P) * D], F, tag="wa")
                nc.sync.dma_start(wa_sb[:].rearrange("p (c o) -> p c o", o=D),
                                  wa.rearrange("(c p) o -> p c o", p=P))
                wb_sb = enc_w.tile([P, (DIN_B // P) * D], F, tag="wb")
                nc.sync.dma_start(wb_sb[:].rearrange("p (c o) -> p c o", o=D),
                                  wb.rearrange("(c p) o -> p c o", p=P))
                _encoder_phase(nc, tc, pools, xa, wa_sb, ba_sb, rwa_sb, rba_sb,
                               DIN_A, zTa)
                _encoder_phase(nc, tc, pools, xb, wb_sb, bb_sb, rwb_sb, rbb_sb,
                               DIN_B, zTb)

            # ---------------- VQ ----------------
            with tc.tile_pool(name="vq_sb", bufs=2) as vq_sb, \
                 tc.tile_pool(name="vq_small", bufs=1) as vq_small, \
                 tc.tile_pool(name="vq_ps", bufs=2, space="PSUM") as vq_ps, \
                 tc.tile_pool(name="vq_w", bufs=1) as vq_w:
                pools.update(vq_sb=vq_sb, vq_ps=vq_ps, vq_small=vq_small)
                e2b = vq_w.tile([P, K], F, tag="e2b")
                nc.sync.dma_start(e2b[:], e2row[0:1, :].to_broadcast([P, K]))
                pools["e2b"] = e2b
                # build ET2 = 2*E^T as 2 chunks of [128, K] (fp32)
                ET2 = [vq_w.tile([P, K], F, tag=f"ET{c}", name=f"ET{c}")
                       for c in range(2)]
                for kt in range(K // P):
                    est = vq_sb.tile([P, D], F, tag="est")
                    nc.sync.dma_start(est[:], Ecb[kt * P:(kt + 1) * P, :])
                    pe = vq_ps.tile([P, D], F, space="PSUM", tag="pz")
                    for ci in range(2):
                        nc.tensor.transpose(out=pe[:, ci * P:(ci + 1) * P],
                                            in_=est[:, ci * P:(ci + 1) * P],
                                            identity=pools["identF"][:])
                    for ci in range(2):
                        nc.scalar.activation(
                            out=ET2[ci][:, kt * P:(kt + 1) * P],
                            in_=pe[:, ci * P:(ci + 1) * P],
                            func=ACTF.Copy, scale=2.0)
                _vq_phase(nc, tc, pools, zTa, ET2, Ecb, za, la, zqTa, ia)
                _vq_phase(nc, tc, pools, zTb, ET2, Ecb, zb, lb, zqTb, ib)

            # ---------------- decoders ----------------
            with tc.tile_pool(name="dec_sb", bufs=2) as dec_sb, \
                 tc.tile_pool(name="dec_ps", bufs=2, space="PSUM") as dec_ps, \
                 tc.tile_pool(name="dec_w", bufs=1) as dec_w:
                pools.update(dec_sb=dec_sb, dec_ps=dec_ps)
                drwa_sb = load_w(drwa.rearrange("r i o -> (r i) o"), NRES * D, D,
                                 "drwa", pool=dec_w)
                drwb_sb = load_w(drwb.rearrange("r i o -> (r i) o"), NRES * D, D,
                                 "drwb", pool=dec_w)
                dwa_sb = dec_w.tile([P, 2 * DIN_A], R, tag="dwa")
                nc.sync.dma_start(dwa_sb[:].rearrange("p (c o) -> p c o", o=DIN_A),
                                  dwa.rearrange("(c p) o -> p c o", p=P))
                dwb_sb = dec_w.tile([P, 2 * DIN_B], R, tag="dwb")
                nc.sync.dma_start(dwb_sb[:].rearrange("p (c o) -> p c o", o=DIN_B),
                                  dwb.rearrange("(c p) o -> p c o", p=P))
                dba_sb = dec_w.tile([1, DIN_A], R, tag="dba")
                nc.sync.dma_start(dba_sb[:], dba[None, :])
                dbb_sb = dec_w.tile([1, DIN_B], R, tag="dbb")
                nc.sync.dma_start(dbb_sb[:], dbb[None, :])
                _decoder_phase(nc, tc, pools, zqTa, drwa_sb, drba_sb, dwa_sb,
                               dba_sb, DIN_A, ra)
                _decoder_phase(nc, tc, pools, zqTb, drwb_sb, drbb_sb, dwb_sb,
                               dbb_sb, DIN_B, rb_o)
                _decoder_phase(nc, tc, pools, zqTb, drwa_sb, drba_sb, dwa_sb,
                               dba_sb, DIN_A, ca)
                _decoder_phase(nc, tc, pools, zqTa, drwb_sb, drbb_sb, dwb_sb,
                               dbb_sb, DIN_B, cb)
    return nc


_PROGRAM = None


def _get_program():
    global _PROGRAM
    if _PROGRAM is None:
        _PROGRAM = _build_program()
    return _PROGRAM


def kernel(x_a, x_b,
           enc_a_w, enc_a_b, enc_a_rw, enc_a_rb,
           enc_b_w, enc_b_b, enc_b_rw, enc_b_rb,
           dec_a_rw, dec_a_rb, dec_a_w, dec_a_b,
           dec_b_rw, dec_b_rb, dec_b_w, dec_b_b,
           codebook):
    nc = _get_program()
    f32 = np.float32
    cb = np.ascontiguousarray(codebook, dtype=f32)
    shared = {
        "wa": np.ascontiguousarray(enc_a_w, f32), "ba": np.ascontiguousarray(enc_a_b, f32),
        "wb": np.ascontiguousarray(enc_b_w, f32), "bb": np.ascontiguousarray(enc_b_b, f32),
        "rwa": np.ascontiguousarray(enc_a_rw, f32), "rba": np.ascontiguousarray(enc_a_rb, f32),
        "rwb": np.ascontiguousarray(enc_b_rw, f32), "rbb": np.ascontiguousarray(enc_b_rb, f32),
        "drwa": np.ascontiguousarray(dec_a_rw, f32), "drba": np.ascontiguousarray(dec_a_rb, f32),
        "drwb": np.ascontiguousarray(dec_b_rw, f32), "drbb": np.ascontiguousarray(dec_b_rb, f32),
        "dwa": np.ascontiguousarray(dec_a_w, f32), "dba": np.ascontiguousarray(dec_a_b, f32),
        "dwb": np.ascontiguousarray(dec_b_w, f32), "dbb": np.ascontiguousarray(dec_b_b, f32),
        "E": cb,
        "e2": np.ascontiguousarray(np.sum(cb * cb, axis=1, dtype=np.float32)[None, :]),
        "identF": np.eye(P, dtype=f32),
        "ones": np.ones((1, P), f32),
    }
    x_a = np.ascontiguousarray(x_a, f32)
    x_b = np.ascontiguousarray(x_b, f32)
    in_maps = []
    for c in range(NCORES):
        m = dict(shared)
        m["xa"] = x_a[c * BC:(c + 1) * BC]
        m["xb"] = x_b[c * BC:(c + 1) * BC]
        in_maps.append(m)
    res = run_bass_kernel_spmd(nc, in_maps, list(range(NCORES)))
    rr = res.results

    def cat(name):
        return np.concatenate([rr[c][name] for c in range(NCORES)], axis=0)

    out = (cat("za"), cat("zb"), cat("la")[:, 0], cat("lb")[:, 0],
           cat("ra"), cat("rb"), cat("ca"), cat("cb"))
    kernel.last_idx = (cat("ia")[:, 0], cat("ib")[:, 0])
    return out


if __name__ == "__main__":
    # tiny self-exercise with random data (shapes only; no reference here)
    rng = np.random.default_rng(0)
    outs = kernel(
        x_a=rng.standard_normal((B, DIN_A)).astype(np.float32),
        x_b=rng.standard_normal((B, DIN_B)).astype(np.float32),
        enc_a_w=rng.standard_normal((DIN_A, D)).astype(np.float32) / 64,
        enc_a_b=np.zeros(D, np.float32),
        enc_a_rw=rng.standard_normal((NRES, D, D)).astype(np.float32) / 16,
        enc_a_rb=np.zeros((NRES, D), np.float32),
        enc_b_w=rng.standard_normal((DIN_B, D)).astype(np.float32) / 45,
        enc_b_b=np.zeros(D, np.float32),
        enc_b_rw=rng.standard_normal((NRES, D, D)).astype(np.float32) / 16,
        enc_b_rb=np.zeros((NRES, D), np.float32),
        dec_a_rw=rng.standard_normal((NRES, D, D)).astype(np.float32) / 16,
        dec_a_rb=np.zeros((NRES, D), np.float32),
        dec_a_w=rng.standard_normal((D, DIN_A)).astype(np.float32) / 16,
        dec_a_b=np.zeros(DIN_A, np.float32),
        dec_b_rw=rng.standard_normal((NRES, D, D)).astype(np.float32) / 16,
        dec_b_rb=np.zeros((NRES, D), np.float32),
        dec_b_w=rng.standard_normal((D, DIN_B)).astype(np.float32) / 16,
        dec_b_b=np.zeros(DIN_B, np.float32),
        codebook=rng.uniform(-1 / K, 1 / K, (K, D)).astype(np.float32),
    )
    for o in outs:
        print(o.shape, o.dtype, float(np.abs(o).mean()))
